# revision 92
# baseline (speedup 1.0000x reference)
"""Multi-head attention (B=2, L=2048, D=1024, H=16) on 8 TRN2 NeuronCores.

Sharding: core c handles batch b=c//4 and heads 4*(c%4) .. 4*(c%4)+3
(tensor-parallel over heads x data-parallel over batch). Each core computes a
partial [L, D] output (its heads' contribution through wo); the host sums the
4 partials per batch and adds bo (+ bv @ wo, folded on host since
sum_k softmax_k * (v+bv) = o + bv).

Device-side structure (per core, 2 head-pairs of 2 heads each):
  - QKV projections in fp8-e4m3 DoubleRow matmuls (2 stacked K=128 blocks per
    instruction at 0.5 cyc/row). Error-compensated 3-term form
    x.w ~= xh.wh + xl.wh + xh.wl with host-prepared hi/lo splits, so accuracy
    stays at bf16 level while the PE cost drops ~2.7x.
  - Q^T/K^T [128 pair-dims, L] bf16; V natural [Lk, 65 per head] with a ones
    column per head so the PV matmul also accumulates the softmax denominator.
  - S^T [Lk, Lq-block] = K^T.T @ Q^T per 128-k-block; two k-blocks share a
    2-bank PSUM tile so one exp activation covers [128, 2, 512].
  - causal: k-blocks above the diagonal skipped; 4 diagonal blocks per
    Lq-block get a multiplicative bf16 mask after exp.
  - PV in O-form: stationary = P chunk [128k, 128q], moving = V+ones
    [128k, 65] -> PSUM [128q, 65] accumulated over k-blocks (cost 65/row vs
    512 for the O^T form). Both heads share one PSUM bank per q-chunk.
  - normalize by the denominator column (reciprocal + per-partition mult),
    PE-transpose the pair-combined [128q, 128d] chunk back to O^T layout for
    the output projection (contract the pair's 128 dims per l-block).
  - scores/exp/PV trimmed on the diagonal blocks (block m only has valid
    queries at q >= 128m in its Lq-block; the mask zeroes the slack).
  - emission weave: projections and the output projection are emitted as
    credit-ordered fillers inside attention so the exp backlog never idles
    the (in-order) PE queue; need-by tags prevent emission-order deadlocks.
"""

from contextlib import ExitStack

import numpy as np

import concourse.bass as bass
import concourse.mybir as mybir
import concourse.tile as tile
from concourse import bacc
from concourse.bass_utils import run_bass_kernel_spmd

B, L, D, H = 2, 2048, 1024, 16
DH = D // H          # 64
P = 128              # partitions
NPAIR = 2            # head pairs per core (4 heads)
LQB = 512            # Lq block
NLQ = L // LQB       # 4
NKB = L // P         # 16 k blocks
KD = D // P          # 8 contraction blocks over D
KK = KD // 2         # 4 DoubleRow pairs
N_CORES = 8

F32 = mybir.dt.float32
BF16 = mybir.dt.bfloat16
FP8 = mybir.dt.float8e4
AF = mybir.ActivationFunctionType
DR = mybir.MatmulPerfMode.DoubleRow

MM_DT = BF16


def build_module(mm_dt=None, iters=1):
    nc = bacc.Bacc("TRN2", target_bir_lowering=False, debug=False,
                   num_devices=N_CORES)

    xt_hi = nc.dram_tensor("xt_hi", [D, L], FP8, kind="ExternalInput").ap()
    xt_lo = nc.dram_tensor("xt_lo", [D, L], FP8, kind="ExternalInput").ap()
    # weights pre-arranged on host to [P, KK, 2, 2P]:
    # w8[p, kk, s, m] = w[(2*kk+s)*128 + p, m]
    w_dram = {}
    for wn in ("wq", "wk", "wv"):
        for part in ("hi", "lo"):
            name = f"{wn}_{part}"
            w_dram[name] = nc.dram_tensor(
                name, [P, KK, 2, 2 * P], FP8, kind="ExternalInput").ap()
    bq = nc.dram_tensor("bq", [2 * P], F32, kind="ExternalInput").ap()
    bk = nc.dram_tensor("bk", [2 * P], F32, kind="ExternalInput").ap()
    wo = nc.dram_tensor("wo", [2 * P, D], BF16, kind="ExternalInput").ap()
    mask = nc.dram_tensor("mask", [4, P, LQB], BF16, kind="ExternalInput").ap()
    ident_d = nc.dram_tensor("ident", [P, P], BF16, kind="ExternalInput").ap()
    out = nc.dram_tensor("out", [L, D], BF16, kind="ExternalOutput").ap()

    with tile.TileContext(nc) as tc, ExitStack() as ctx:
        ctx.enter_context(
            nc.allow_low_precision(reason="fp8/bf16 matmul inputs"))
        consts = ctx.enter_context(tc.tile_pool(name="consts", bufs=1))
        proj_sb = ctx.enter_context(tc.tile_pool(name="proj_sb", bufs=1))
        ptp = ctx.enter_context(tc.tile_pool(name="ptp", bufs=24))
        nrm = ctx.enter_context(tc.tile_pool(name="nrm", bufs=8))
        ps = ctx.enter_context(tc.tile_pool(name="ps", bufs=1, space="PSUM"))

        # ---- load constants --------------------------------------------
        # DMA issue order is the startup critical path (shared HWDGE takes
        # ~650ns per DMA, transfers ~728ns per 256KB): wq first, then x
        # hi/lo k-pairs in the order the kk-outer projection groups consume
        # them, with wk/wv slotted between; everything not needed by the
        # first groups goes last.
        w_sb = {}
        for name in w_dram:
            w_sb[name] = consts.tile([P, KK, 2, 2 * P], FP8, tag=name,
                                     name=name)
        xh_sb = consts.tile([P, KD, L], FP8, tag="xh")
        xl_sb = consts.tile([P, KD, L], FP8, tag="xl")

        bq_sb = consts.tile([P, NPAIR], F32, tag="bq")
        bk_sb = consts.tile([P, NPAIR], F32, tag="bk")
        nc.sync.dma_start(out=w_sb["wq_hi"][:], in_=w_dram["wq_hi"])
        nc.sync.dma_start(out=w_sb["wq_lo"][:], in_=w_dram["wq_lo"])
        for k in range(KD):
            nc.sync.dma_start(out=xh_sb[:, k, :], in_=xt_hi[k * P:(k + 1) * P, :])
            nc.sync.dma_start(out=xl_sb[:, k, :], in_=xt_lo[k * P:(k + 1) * P, :])
            if k == 1:
                nc.sync.dma_start(out=w_sb["wk_hi"][:], in_=w_dram["wk_hi"])
                nc.sync.dma_start(out=w_sb["wk_lo"][:], in_=w_dram["wk_lo"])
            elif k == 3:
                nc.sync.dma_start(out=w_sb["wv_hi"][:], in_=w_dram["wv_hi"])
                nc.sync.dma_start(out=w_sb["wv_lo"][:], in_=w_dram["wv_lo"])
            elif k == 5:
                for p in range(NPAIR):
                    nc.sync.dma_start(
                        out=bq_sb[:, p:p + 1],
                        in_=bq[p * P:(p + 1) * P].rearrange("(p o) -> p o", o=1))
                    nc.sync.dma_start(
                        out=bk_sb[:, p:p + 1],
                        in_=bk[p * P:(p + 1) * P].rearrange("(p o) -> p o", o=1))

        mask_sb = consts.tile([P, 4, LQB], BF16, tag="mask")
        for m in range(4):
            nc.sync.dma_start(out=mask_sb[:, m, :], in_=mask[m])

        ident = consts.tile([P, P], BF16, tag="ident")
        nc.sync.dma_start(out=ident[:], in_=ident_d)

        wo_sb = []
        for p in range(NPAIR):
            t = consts.tile([P, D], BF16, tag=f"wo{p}", name=f"wo{p}")
            nc.sync.dma_start(out=t[:], in_=wo[p * P:(p + 1) * P, :])
            wo_sb.append(t)

        # ---- per-pair persistent tiles ---------------------------------
        qt_t, kt_t, vx_t, ot_t = [], [], [], []
        for p in range(NPAIR):
            qt_t.append(proj_sb.tile([P, L], BF16, tag=f"qt{p}", name=f"qt{p}"))
            kt_t.append(proj_sb.tile([P, L], BF16, tag=f"kt{p}", name=f"kt{p}"))
            vx_t.append(proj_sb.tile([P, NKB, 2, DH + 1], BF16, tag=f"vx{p}",
                                     name=f"vx{p}"))
            ot_t.append(proj_sb.tile([P, L], BF16, tag=f"ot{p}", name=f"ot{p}"))

        # ---- emission closures -----------------------------------------
        # 3-term compensated fp8 DoubleRow accumulation:
        #   (stat_hi, mov_hi), (stat_lo, mov_hi), (stat_hi, mov_lo)
        def dr_group(acc, stats, movs):
            sh, sl = stats
            mh, ml = movs
            n = 3 * KK
            i = 0
            # kk outer so the startup groups consume x k-pairs in DMA order
            for kk in range(KK):
                for st, mv in ((sh, mh), (sl, mh), (sh, ml)):
                    nc.tensor.matmul(acc, st(kk), mv(kk),
                                     start=(i == 0), stop=(i == n - 1),
                                     perf_mode=DR)
                    i += 1

        def qk_group(p, dst, wname, b_sb, c):
            acc = ps.tile([P, LQB], F32, tag="proj", bufs=2, name="acc_qk")
            dr_group(
                acc[:],
                (lambda kk, n=wname: w_sb[n + "_hi"][:, kk, :, p * P:(p + 1) * P],
                 lambda kk, n=wname: w_sb[n + "_lo"][:, kk, :, p * P:(p + 1) * P]),
                (lambda kk: xh_sb[:, 2 * kk:2 * kk + 2, c * LQB:(c + 1) * LQB],
                 lambda kk: xl_sb[:, 2 * kk:2 * kk + 2, c * LQB:(c + 1) * LQB]))
            nc.vector.tensor_scalar_add(
                dst[:, c * LQB:(c + 1) * LQB], acc[:], b_sb[:, p:p + 1])

        def v_group(p, j):
            acc = ps.tile([P, 2, DH], F32, tag="proj", bufs=2, name="acc_v")
            dr_group(
                acc[:],
                (lambda kk: xh_sb[:, 2 * kk:2 * kk + 2, j * P:(j + 1) * P],
                 lambda kk: xl_sb[:, 2 * kk:2 * kk + 2, j * P:(j + 1) * P]),
                (lambda kk: w_sb["wv_hi"][:, kk, :, p * P:(p + 1) * P],
                 lambda kk: w_sb["wv_lo"][:, kk, :, p * P:(p + 1) * P]))
            nc.vector.tensor_copy(vx_t[p][:, j, :, 0:DH], acc[:])

        def qk_pair(p, c):
            qk_group(p, qt_t[p], "wq", bq_sb, c)
            qk_group(p, kt_t[p], "wk", bk_sb, c)

        # attention for pair p, Lq-block i: scores+exp (both heads), then
        # PV chunks + normalize + transpose. Yields (pe_cost_us, act_cost_us,
        # closure) so the weave can interleave fillers.
        def attn_units(p, i):
            njb = 4 * i + 4
            pts = {}

            def scores_pair(h, jj):
                s2 = ps.tile([P, 2, LQB], F32, tag="s", bufs=2, name="s2")
                pt = ptp.tile([P, 2, LQB], BF16, tag="pt", name="pt")
                pts[(h, 2 * jj)] = (pt, 0)
                pts[(h, 2 * jj + 1)] = (pt, 1)
                hp = h * DH
                for s in range(2):
                    jb = 2 * jj + s
                    # diagonal block m has valid queries only at q >= 128m
                    # within this Lq-block: trim the matmul to that range
                    q0 = max(0, (jb - 4 * i) * P)
                    nc.tensor.matmul(
                        s2[:, s, q0:LQB],
                        kt_t[p][hp:hp + DH, jb * P:(jb + 1) * P],
                        qt_t[p][hp:hp + DH, i * LQB + q0:(i + 1) * LQB],
                        start=True, stop=True)
                # qt/kt carry a 256x host-side scale each (fp8 ranges), so
                # scores are 65536x; fold the correction into the exp scale.
                # The exp (and mask) cover the union of both slots' valid
                # ranges; the mask zeroes the slack columns (the tiny scale
                # keeps exp of stale PSUM finite).
                q0e = max(0, (2 * jj - 4 * i) * P)
                nc.scalar.activation(pt[:, :, q0e:LQB], s2[:, :, q0e:LQB],
                                     AF.Exp,
                                     scale=1.0 / (np.sqrt(DH) * 65536.0))
                if jj >= 2 * i:
                    m0 = 2 * (jj - 2 * i)
                    # the first diagonal (jj == 2i, emitted first, most slack
                    # before PV) goes to the otherwise-idle Pool when i >= 1
                    eng = nc.gpsimd if (jj == 2 * i and i >= 1) else nc.vector
                    eng.tensor_mul(pt[:, :, q0e:LQB], pt[:, :, q0e:LQB],
                                   mask_sb[:, m0:m0 + 2, q0e:LQB])

            def pv_chunk(c):
                pv = ps.tile([P, 2, DH + 1], F32, tag="pv", bufs=2, name="pv",
                             padded_shape=[P, 2, 256])
                # diagonal blocks with m > c are fully masked in this chunk
                jbs = [jb for jb in range(njb) if jb - 4 * i <= c]
                n = 2 * len(jbs)
                idx = 0
                for h in range(2):
                    for jb in jbs:
                        pt, s = pts[(h, jb)]
                        nc.tensor.matmul(
                            pv[:, h, :],
                            pt[:, s, c * P:(c + 1) * P],
                            vx_t[p][:, jb, h, :],
                            start=(idx == 0), stop=(idx == n - 1))
                        idx += 1
                rec = nrm.tile([P, 2], F32, tag="rec", name="rec")
                nc.vector.reciprocal(rec[:], pv[:, :, DH])
                pre = nrm.tile([P, P], BF16, tag="pre", name="pre")
                for h in range(2):
                    nc.vector.tensor_scalar_mul(
                        pre[:, h * DH:(h + 1) * DH],
                        pv[:, h, 0:DH],
                        rec[:, h:h + 1])
                tr = ps.tile([P, P], BF16, tag="pv", bufs=2, name="tr",
                             padded_shape=[P, 1024])
                nc.tensor.matmul(tr[:], pre[:], ident[:], is_transpose=True)
                nc.vector.tensor_copy(
                    ot_t[p][:, i * LQB + c * P:i * LQB + (c + 1) * P], tr[:])

            # diagonal jj first so their exp+mask complete well before the
            # PV groups (whose last k-blocks are the masked ones) need them
            jjs = [2 * i, 2 * i + 1] + list(range(2 * i))
            for jj in jjs:
                w0 = LQB - max(0, (2 * jj - 4 * i) * P)
                w1 = LQB - max(0, (2 * jj + 1 - 4 * i) * P)
                pe_us = (w0 + w1) * 0.4167e-3 + 0.01
                act_us = 2 * w0 * 0.833e-3 + 0.25
                for h in range(2):
                    yield ("sc", pe_us, act_us,
                           lambda hh=h, j=jj: scores_pair(hh, j))
            for c in range(NLQ):
                yield (("pv", c), 0.055 * (4 * i + c + 1) + 0.06, 0.0,
                       lambda cc=c: pv_chunk(cc))

        def outproj_half(l, half):
            acc = ps.tile([P, LQB], F32, tag="proj", bufs=2, name="acc_o")
            for p in range(NPAIR):
                nc.tensor.matmul(
                    acc[:],
                    ot_t[p][:, l * P:(l + 1) * P],
                    wo_sb[p][:, half * LQB:(half + 1) * LQB],
                    start=(p == 0), stop=(p == NPAIR - 1))
            osb = nrm.tile([P, LQB], BF16, tag="osb", bufs=4, name="osb")
            # the last l-blocks run after attention: alternate the
            # idle ACT and DVE engines so the drain copies overlap
            if l >= 12 and half == 1:
                nc.scalar.copy(osb[:], acc[:])
            else:
                nc.vector.tensor_copy(osb[:], acc[:])
            nc.sync.dma_start(
                out=out[l * P:(l + 1) * P, half * LQB:(half + 1) * LQB],
                in_=osb[:])

        # ---- weave ------------------------------------------------------
        # Attention (p, i) only needs q/k c-chunks <= i and V j-blocks
        # <= 4i+3, so almost all projection work can fill the PE bubbles
        # that the exp backlog creates inside attention. Emit only the
        # minimum up front, start attention early, and feed the rest (plus
        # the output projection) as credit-ordered fillers.

        # warm the exp table on ACT while the input DMAs stream
        scr = consts.tile([P, 1], F32, tag="scr")
        nc.vector.memset(scr[:], 0.0)
        scr2 = consts.tile([P, 1], F32, tag="scr2")
        nc.scalar.activation(scr2[:], scr[:], AF.Exp, scale=1.0)

        nc.vector.memset(vx_t[0][:, :, :, DH:DH + 1], 1.0)

        qk_pair(0, 0)
        for j in range(4):
            v_group(0, j)

        # fillers for the attention weave: (pe_us, need, fn) where need =
        # (phase, i) means the filler MUST be emitted before attention
        # phase p reaches Lq-block i — PE is in-order, so a consumer
        # emitted ahead of its producer would deadlock.
        fillers = []
        for c in range(1, NLQ):
            fillers.append((2.56, (0, c), lambda cc=c: qk_pair(0, cc)))
            for j in range(4 * c, 4 * c + 4):
                fillers.append((0.32, (0, c), lambda jj=j: v_group(0, jj)))
        fillers.append(
            (0.1, (1, 0),
             lambda: nc.vector.memset(vx_t[1][:, :, :, DH:DH + 1], 1.0)))
        for c in range(NLQ):
            fillers.append((2.56, (1, c), lambda cc=c: qk_pair(1, cc)))
            for j in range(4 * c, 4 * c + 4):
                fillers.append((0.32, (1, c), lambda jj=j: v_group(1, jj)))

        credit = 0.0

        def pop_filler():
            nonlocal credit
            f_pe, _need, f_fn = fillers.pop(0)
            f_fn()
            credit -= f_pe

        def force_due(phase, i):
            # emit every filler still pending that attn(phase, i) depends on
            rest = []
            for f in fillers:
                need = f[1]
                if need is not None and need <= (phase, i):
                    f[2]()
                else:
                    rest.append(f)
            fillers[:] = rest

        for i in range(NLQ):
            force_due(0, i)
            for kind, pe_us, act_us, fn in attn_units(0, i):
                fn()
                credit += act_us - pe_us
                while credit > 0.2 and fillers:
                    pop_filler()
        # pair1 attention; leftover proj fillers first, then output
        # projection l-blocks as their ot chunks (both pairs) complete.
        credit = 0.0
        for i in range(NLQ):
            force_due(1, i)
            for kind, pe_us, act_us, fn in attn_units(1, i):
                fn()
                if isinstance(kind, tuple) and kind[0] == "pv":
                    for hf in range(2):
                        fillers.append(
                            (0.43, None,
                             lambda ll=4 * i + kind[1], h=hf:
                                 outproj_half(ll, h)))
                    if i == NLQ - 1:
                        # keep the tail short: drain outproj during the
                        # last i-block regardless of ACT credit
                        for _ in range(5):
                            if len(fillers) > 2:
                                pop_filler()
                credit += act_us - pe_us
                # reserve a few outproj groups so the last i-block's exp
                # backlog (no new fillers arrive there) still has PE cover
                reserve = 18 if i < NLQ - 1 else 0
                while credit > 0.2 and len(fillers) > reserve:
                    pop_filler()
        for _pe, _need, f_fn in fillers:
            f_fn()

    nc.compile()
    return nc


_CACHE = {}


def _get_nc(mm_dt=None, iters=1):
    key = (str(mm_dt), iters)
    if key not in _CACHE:
        _CACHE[key] = build_module(mm_dt, iters)
    return _CACHE[key]


def _np_fp8():
    import ml_dtypes
    return ml_dtypes.float8_e4m3


def _np_bf16():
    import ml_dtypes
    return ml_dtypes.bfloat16


def _make_in_maps(x, causal_mask, wq, bq, wk, bk, wv, bv, wo):
    fp8 = _np_fp8()
    bf16 = _np_bf16()
    x = np.asarray(x, np.float32)
    cm = np.asarray(causal_mask)
    # 4 multiplicative mask tiles [128, 512]: tile m covers k-block j=4i+m
    # within Lq-block i -> tile[p, c] = 0 if mask(q=c, k=128m+p) else 1
    mt = np.empty((4, P, LQB), np.float32)
    for m in range(4):
        mt[m] = (~cm[0, 0, 0:LQB, m * P:(m + 1) * P]).T.astype(np.float32)
    mt = mt.astype(bf16)
    ident = np.eye(P, dtype=bf16)

    # fp8 e4m3 has its min normal at 2^-6; scale x by 8 and w by 32 so the
    # hi/lo residuals stay out of the subnormal floor (verified: unscaled
    # 3-term compensation is ~3e-2, scaled ~1.3e-3). The combined 256x is
    # folded into bq/bk (x256), the exp scale (/65536), and wo (/256).
    def split8(a, s):
        a = np.ascontiguousarray(a) * s
        hi = a.astype(fp8)
        lo = (a - hi.astype(np.float32)).astype(fp8)
        return hi, lo

    def warr(w):
        # [D, 2P] -> [P, KK, 2, 2P]
        return np.ascontiguousarray(
            w.reshape(KK, 2, P, 2 * P).transpose(2, 0, 1, 3))

    in_maps = []
    for c in range(N_CORES):
        b = c // 4
        g = c % 4
        cols = slice(2 * P * g, 2 * P * (g + 1))
        xt = np.ascontiguousarray(x[b].T)
        xt_hi, xt_lo = split8(xt, 8.0)
        m = {"xt_hi": xt_hi, "xt_lo": xt_lo,
             "bq": np.ascontiguousarray(np.asarray(bq, np.float32)[cols]) * 256.0,
             "bk": np.ascontiguousarray(np.asarray(bk, np.float32)[cols]) * 256.0,
             "wo": np.ascontiguousarray(
                 np.asarray(wo, np.float32)[cols, :] / 256.0).astype(bf16),
             "mask": mt, "ident": ident}
        for wn, w in (("wq", wq), ("wk", wk), ("wv", wv)):
            wc = np.asarray(w, np.float32)[:, cols]
            hi, lo = split8(wc, 32.0)
            m[f"{wn}_hi"] = warr(hi)
            m[f"{wn}_lo"] = warr(lo)
        in_maps.append(m)
    return in_maps


def run(inputs, trace=False, mm_dt=None, iters=1, **kw):
    nc = _get_nc(mm_dt, iters)
    in_maps = _make_in_maps(
        inputs["x"], inputs["causal_mask"], inputs["wq"], inputs["bq"],
        inputs["wk"], inputs["bk"], inputs["wv"], inputs["bv"], inputs["wo"])
    res = run_bass_kernel_spmd(nc, in_maps, list(range(N_CORES)),
                               trace=trace, **kw)
    bo = np.asarray(inputs["bo"], np.float32)
    bv = np.asarray(inputs["bv"], np.float32)
    wo_f = np.asarray(inputs["wo"], np.float32)
    out = np.zeros((B, L, D), np.float32)
    for c in range(N_CORES):
        out[c // 4] += res.results[c]["out"].astype(np.float32)
    out += (bo + bv @ wo_f)[None, None, :]
    return out, res


def kernel(**inputs):
    out, _ = run(inputs)
    return out


# revision 94
# speedup vs baseline: 1.0026x; 1.0026x over previous
"""Multi-head attention (B=2, L=2048, D=1024, H=16) on 8 TRN2 NeuronCores.

Sharding: core c handles batch b=c//4 and heads 4*(c%4) .. 4*(c%4)+3
(tensor-parallel over heads x data-parallel over batch). Each core computes a
partial [L, D] output (its heads' contribution through wo); the host sums the
4 partials per batch and adds bo (+ bv @ wo, folded on host since
sum_k softmax_k * (v+bv) = o + bv).

Device-side structure (per core, 2 head-pairs of 2 heads each):
  - QKV projections in fp8-e4m3 DoubleRow matmuls (2 stacked K=128 blocks per
    instruction at 0.5 cyc/row). Error-compensated 3-term form
    x.w ~= xh.wh + xl.wh + xh.wl with host-prepared hi/lo splits, so accuracy
    stays at bf16 level while the PE cost drops ~2.7x.
  - Q^T/K^T [128 pair-dims, L] bf16; V natural [Lk, 65 per head] with a ones
    column per head so the PV matmul also accumulates the softmax denominator.
  - S^T [Lk, Lq-block] = K^T.T @ Q^T per 128-k-block; two k-blocks share a
    2-bank PSUM tile so one exp activation covers [128, 2, 512].
  - causal: k-blocks above the diagonal skipped; 4 diagonal blocks per
    Lq-block get a multiplicative bf16 mask after exp.
  - PV in O-form: stationary = P chunk [128k, 128q], moving = V+ones
    [128k, 65] -> PSUM [128q, 65] accumulated over k-blocks (cost 65/row vs
    512 for the O^T form). Both heads share one PSUM bank per q-chunk.
  - normalize by the denominator column (reciprocal + per-partition mult),
    PE-transpose the pair-combined [128q, 128d] chunk back to O^T layout for
    the output projection (contract the pair's 128 dims per l-block).
  - scores/exp/PV trimmed on the diagonal blocks (block m only has valid
    queries at q >= 128m in its Lq-block; the mask zeroes the slack).
  - emission weave: projections and the output projection are emitted as
    credit-ordered fillers inside attention so the exp backlog never idles
    the (in-order) PE queue; need-by tags prevent emission-order deadlocks.
"""

from contextlib import ExitStack

import numpy as np

import concourse.bass as bass
import concourse.mybir as mybir
import concourse.tile as tile
from concourse import bacc
from concourse.bass_utils import run_bass_kernel_spmd

B, L, D, H = 2, 2048, 1024, 16
DH = D // H          # 64
P = 128              # partitions
NPAIR = 2            # head pairs per core (4 heads)
LQB = 512            # Lq block
NLQ = L // LQB       # 4
NKB = L // P         # 16 k blocks
KD = D // P          # 8 contraction blocks over D
KK = KD // 2         # 4 DoubleRow pairs
N_CORES = 8

F32 = mybir.dt.float32
BF16 = mybir.dt.bfloat16
FP8 = mybir.dt.float8e4
AF = mybir.ActivationFunctionType
DR = mybir.MatmulPerfMode.DoubleRow

MM_DT = BF16


def build_module(mm_dt=None, iters=1):
    nc = bacc.Bacc("TRN2", target_bir_lowering=False, debug=False,
                   num_devices=N_CORES)

    xt_hi = nc.dram_tensor("xt_hi", [D, L], FP8, kind="ExternalInput").ap()
    xt_lo = nc.dram_tensor("xt_lo", [D, L], FP8, kind="ExternalInput").ap()
    # weights pre-arranged on host to [P, KK, 2, 2P]:
    # w8[p, kk, s, m] = w[(2*kk+s)*128 + p, m]
    w_dram = {}
    for wn in ("wq", "wk", "wv"):
        for part in ("hi", "lo"):
            name = f"{wn}_{part}"
            w_dram[name] = nc.dram_tensor(
                name, [P, KK, 2, 2 * P], FP8, kind="ExternalInput").ap()
    bq = nc.dram_tensor("bq", [2 * P], F32, kind="ExternalInput").ap()
    bk = nc.dram_tensor("bk", [2 * P], F32, kind="ExternalInput").ap()
    wo = nc.dram_tensor("wo", [2 * P, D], BF16, kind="ExternalInput").ap()
    mask = nc.dram_tensor("mask", [4, P, LQB], BF16, kind="ExternalInput").ap()
    ident_d = nc.dram_tensor("ident", [P, P], BF16, kind="ExternalInput").ap()
    out = nc.dram_tensor("out", [L, D], BF16, kind="ExternalOutput").ap()

    with tile.TileContext(nc) as tc, ExitStack() as ctx:
        ctx.enter_context(
            nc.allow_low_precision(reason="fp8/bf16 matmul inputs"))
        consts = ctx.enter_context(tc.tile_pool(name="consts", bufs=1))
        proj_sb = ctx.enter_context(tc.tile_pool(name="proj_sb", bufs=1))
        ptp = ctx.enter_context(tc.tile_pool(name="ptp", bufs=24))
        nrm = ctx.enter_context(tc.tile_pool(name="nrm", bufs=8))
        ps = ctx.enter_context(tc.tile_pool(name="ps", bufs=1, space="PSUM"))

        # ---- load constants --------------------------------------------
        # DMA issue order is the startup critical path (shared HWDGE takes
        # ~650ns per DMA, transfers ~728ns per 256KB): wq first, then x
        # hi/lo k-pairs in the order the kk-outer projection groups consume
        # them, with wk/wv slotted between; everything not needed by the
        # first groups goes last.
        w_sb = {}
        for name in w_dram:
            w_sb[name] = consts.tile([P, KK, 2, 2 * P], FP8, tag=name,
                                     name=name)
        xh_sb = consts.tile([P, KD, L], FP8, tag="xh")
        xl_sb = consts.tile([P, KD, L], FP8, tag="xl")

        bq_sb = consts.tile([P, NPAIR], F32, tag="bq")
        bk_sb = consts.tile([P, NPAIR], F32, tag="bk")
        nc.sync.dma_start(out=w_sb["wq_hi"][:], in_=w_dram["wq_hi"])
        nc.sync.dma_start(out=w_sb["wq_lo"][:], in_=w_dram["wq_lo"])
        for k in range(KD):
            nc.sync.dma_start(out=xh_sb[:, k, :], in_=xt_hi[k * P:(k + 1) * P, :])
            nc.sync.dma_start(out=xl_sb[:, k, :], in_=xt_lo[k * P:(k + 1) * P, :])
            if k == 1:
                nc.sync.dma_start(out=w_sb["wk_hi"][:], in_=w_dram["wk_hi"])
                nc.sync.dma_start(out=w_sb["wk_lo"][:], in_=w_dram["wk_lo"])
            elif k == 3:
                nc.sync.dma_start(out=w_sb["wv_hi"][:], in_=w_dram["wv_hi"])
                nc.sync.dma_start(out=w_sb["wv_lo"][:], in_=w_dram["wv_lo"])
            elif k == 5:
                for p in range(NPAIR):
                    nc.sync.dma_start(
                        out=bq_sb[:, p:p + 1],
                        in_=bq[p * P:(p + 1) * P].rearrange("(p o) -> p o", o=1))
                    nc.sync.dma_start(
                        out=bk_sb[:, p:p + 1],
                        in_=bk[p * P:(p + 1) * P].rearrange("(p o) -> p o", o=1))

        mask_sb = consts.tile([P, 4, LQB], BF16, tag="mask")
        for m in range(4):
            nc.sync.dma_start(out=mask_sb[:, m, :], in_=mask[m])

        ident = consts.tile([P, P], BF16, tag="ident")
        nc.sync.dma_start(out=ident[:], in_=ident_d)

        wo_sb = []
        for p in range(NPAIR):
            t = consts.tile([P, D], BF16, tag=f"wo{p}", name=f"wo{p}")
            nc.sync.dma_start(out=t[:], in_=wo[p * P:(p + 1) * P, :])
            wo_sb.append(t)

        # ---- per-pair persistent tiles ---------------------------------
        qt_t, kt_t, vx_t, ot_t = [], [], [], []
        for p in range(NPAIR):
            qt_t.append(proj_sb.tile([P, L], BF16, tag=f"qt{p}", name=f"qt{p}"))
            kt_t.append(proj_sb.tile([P, L], BF16, tag=f"kt{p}", name=f"kt{p}"))
            vx_t.append(proj_sb.tile([P, NKB, 2, DH + 1], BF16, tag=f"vx{p}",
                                     name=f"vx{p}"))
            ot_t.append(proj_sb.tile([P, L], BF16, tag=f"ot{p}", name=f"ot{p}"))

        # ---- emission closures -----------------------------------------
        # 3-term compensated fp8 DoubleRow accumulation:
        #   (stat_hi, mov_hi), (stat_lo, mov_hi), (stat_hi, mov_lo)
        def dr_group(acc, stats, movs):
            sh, sl = stats
            mh, ml = movs
            n = 3 * KK
            i = 0
            # kk outer so the startup groups consume x k-pairs in DMA order
            for kk in range(KK):
                for st, mv in ((sh, mh), (sl, mh), (sh, ml)):
                    nc.tensor.matmul(acc, st(kk), mv(kk),
                                     start=(i == 0), stop=(i == n - 1),
                                     perf_mode=DR)
                    i += 1

        def qk_group(p, dst, wname, b_sb, c):
            acc = ps.tile([P, LQB], F32, tag="proj", bufs=2, name="acc_qk")
            dr_group(
                acc[:],
                (lambda kk, n=wname: w_sb[n + "_hi"][:, kk, :, p * P:(p + 1) * P],
                 lambda kk, n=wname: w_sb[n + "_lo"][:, kk, :, p * P:(p + 1) * P]),
                (lambda kk: xh_sb[:, 2 * kk:2 * kk + 2, c * LQB:(c + 1) * LQB],
                 lambda kk: xl_sb[:, 2 * kk:2 * kk + 2, c * LQB:(c + 1) * LQB]))
            nc.vector.tensor_scalar_add(
                dst[:, c * LQB:(c + 1) * LQB], acc[:], b_sb[:, p:p + 1])

        def v_group(p, j):
            acc = ps.tile([P, 2, DH], F32, tag="proj", bufs=2, name="acc_v")
            dr_group(
                acc[:],
                (lambda kk: xh_sb[:, 2 * kk:2 * kk + 2, j * P:(j + 1) * P],
                 lambda kk: xl_sb[:, 2 * kk:2 * kk + 2, j * P:(j + 1) * P]),
                (lambda kk: w_sb["wv_hi"][:, kk, :, p * P:(p + 1) * P],
                 lambda kk: w_sb["wv_lo"][:, kk, :, p * P:(p + 1) * P]))
            nc.vector.tensor_copy(vx_t[p][:, j, :, 0:DH], acc[:])

        def qk_pair(p, c):
            qk_group(p, qt_t[p], "wq", bq_sb, c)
            qk_group(p, kt_t[p], "wk", bk_sb, c)

        # attention for pair p, Lq-block i: scores+exp (both heads), then
        # PV chunks + normalize + transpose. Yields (pe_cost_us, act_cost_us,
        # closure) so the weave can interleave fillers.
        def attn_units(p, i):
            njb = 4 * i + 4
            pts = {}

            def scores_pair(h, jj):
                s2 = ps.tile([P, 2, LQB], F32, tag="s", bufs=2, name="s2")
                pt = ptp.tile([P, 2, LQB], BF16, tag="pt", name="pt")
                pts[(h, 2 * jj)] = (pt, 0)
                pts[(h, 2 * jj + 1)] = (pt, 1)
                hp = h * DH
                for s in range(2):
                    jb = 2 * jj + s
                    # diagonal block m has valid queries only at q >= 128m
                    # within this Lq-block: trim the matmul to that range
                    q0 = max(0, (jb - 4 * i) * P)
                    nc.tensor.matmul(
                        s2[:, s, q0:LQB],
                        kt_t[p][hp:hp + DH, jb * P:(jb + 1) * P],
                        qt_t[p][hp:hp + DH, i * LQB + q0:(i + 1) * LQB],
                        start=True, stop=True)
                # qt/kt carry a 256x host-side scale each (fp8 ranges), so
                # scores are 65536x; fold the correction into the exp scale.
                # The exp (and mask) cover the union of both slots' valid
                # ranges; the mask zeroes the slack columns (the tiny scale
                # keeps exp of stale PSUM finite).
                q0e = max(0, (2 * jj - 4 * i) * P)
                nc.scalar.activation(pt[:, :, q0e:LQB], s2[:, :, q0e:LQB],
                                     AF.Exp,
                                     scale=1.0 / (np.sqrt(DH) * 65536.0))
                if jj >= 2 * i:
                    m0 = 2 * (jj - 2 * i)
                    # the first diagonal (jj == 2i, emitted first, most slack
                    # before PV) goes to the otherwise-idle Pool when i >= 1
                    eng = nc.gpsimd if (jj == 2 * i and i >= 1) else nc.vector
                    eng.tensor_mul(pt[:, :, q0e:LQB], pt[:, :, q0e:LQB],
                                   mask_sb[:, m0:m0 + 2, q0e:LQB])

            def pv_chunk(c):
                # full-bank tile: cols 0..64 per head hold PV+denominator,
                # spare bytes of the second bank hold the transpose output
                # (no separate slot, so pv slots rotate two chunks apart and
                # the reciprocal+normalize round-trip stays off the PE path)
                pv = ps.tile([P, 2, 256], F32, tag="pv", bufs=2, name="pv")
                # diagonal blocks with m > c are fully masked in this chunk
                jbs = [jb for jb in range(njb) if jb - 4 * i <= c]
                n = 2 * len(jbs)
                idx = 0
                for h in range(2):
                    for jb in jbs:
                        pt, s = pts[(h, jb)]
                        nc.tensor.matmul(
                            pv[:, h, 0:DH + 1],
                            pt[:, s, c * P:(c + 1) * P],
                            vx_t[p][:, jb, h, :],
                            start=(idx == 0), stop=(idx == n - 1))
                        idx += 1
                rec = nrm.tile([P, 2], F32, tag="rec", name="rec")
                nc.vector.reciprocal(rec[:], pv[:, :, DH])
                pre = nrm.tile([P, P], BF16, tag="pre", name="pre")
                for h in range(2):
                    nc.vector.tensor_scalar_mul(
                        pre[:, h * DH:(h + 1) * DH],
                        pv[:, h, 0:DH],
                        rec[:, h:h + 1])
                tr = pv[:, 1, 128:192].bitcast(BF16)
                nc.tensor.matmul(tr, pre[:], ident[:], is_transpose=True)
                nc.vector.tensor_copy(
                    ot_t[p][:, i * LQB + c * P:i * LQB + (c + 1) * P], tr)

            # diagonal jj first so their exp+mask complete well before the
            # PV groups (whose last k-blocks are the masked ones) need them
            jjs = [2 * i, 2 * i + 1] + list(range(2 * i))
            for jj in jjs:
                w0 = LQB - max(0, (2 * jj - 4 * i) * P)
                w1 = LQB - max(0, (2 * jj + 1 - 4 * i) * P)
                pe_us = (w0 + w1) * 0.4167e-3 + 0.01
                act_us = 2 * w0 * 0.833e-3 + 0.25
                for h in range(2):
                    yield ("sc", pe_us, act_us,
                           lambda hh=h, j=jj: scores_pair(hh, j))
            for c in range(NLQ):
                yield (("pv", c), 0.055 * (4 * i + c + 1) + 0.06, 0.0,
                       lambda cc=c: pv_chunk(cc))

        def outproj_half(l, half):
            acc = ps.tile([P, LQB], F32, tag="proj", bufs=2, name="acc_o")
            for p in range(NPAIR):
                nc.tensor.matmul(
                    acc[:],
                    ot_t[p][:, l * P:(l + 1) * P],
                    wo_sb[p][:, half * LQB:(half + 1) * LQB],
                    start=(p == 0), stop=(p == NPAIR - 1))
            osb = nrm.tile([P, LQB], BF16, tag="osb", bufs=4, name="osb")
            # the last l-blocks run after attention: alternate the
            # idle ACT and DVE engines so the drain copies overlap
            if l >= 12 and half == 1:
                nc.scalar.copy(osb[:], acc[:])
            else:
                nc.vector.tensor_copy(osb[:], acc[:])
            nc.sync.dma_start(
                out=out[l * P:(l + 1) * P, half * LQB:(half + 1) * LQB],
                in_=osb[:])

        # ---- weave ------------------------------------------------------
        # Attention (p, i) only needs q/k c-chunks <= i and V j-blocks
        # <= 4i+3, so almost all projection work can fill the PE bubbles
        # that the exp backlog creates inside attention. Emit only the
        # minimum up front, start attention early, and feed the rest (plus
        # the output projection) as credit-ordered fillers.

        # warm the exp table on ACT while the input DMAs stream
        scr = consts.tile([P, 1], F32, tag="scr")
        nc.vector.memset(scr[:], 0.0)
        scr2 = consts.tile([P, 1], F32, tag="scr2")
        nc.scalar.activation(scr2[:], scr[:], AF.Exp, scale=1.0)

        nc.vector.memset(vx_t[0][:, :, :, DH:DH + 1], 1.0)

        qk_pair(0, 0)
        for j in range(4):
            v_group(0, j)

        # fillers for the attention weave: (pe_us, need, fn) where need =
        # (phase, i) means the filler MUST be emitted before attention
        # phase p reaches Lq-block i — PE is in-order, so a consumer
        # emitted ahead of its producer would deadlock.
        fillers = []
        for c in range(1, NLQ):
            fillers.append((2.56, (0, c), lambda cc=c: qk_pair(0, cc)))
            for j in range(4 * c, 4 * c + 4):
                fillers.append((0.32, (0, c), lambda jj=j: v_group(0, jj)))
        fillers.append(
            (0.1, (1, 0),
             lambda: nc.vector.memset(vx_t[1][:, :, :, DH:DH + 1], 1.0)))
        for c in range(NLQ):
            fillers.append((2.56, (1, c), lambda cc=c: qk_pair(1, cc)))
            for j in range(4 * c, 4 * c + 4):
                fillers.append((0.32, (1, c), lambda jj=j: v_group(1, jj)))

        credit = 0.0

        def pop_filler():
            nonlocal credit
            f_pe, _need, f_fn = fillers.pop(0)
            f_fn()
            credit -= f_pe

        def force_due(phase, i):
            # emit every filler still pending that attn(phase, i) depends on
            rest = []
            for f in fillers:
                need = f[1]
                if need is not None and need <= (phase, i):
                    f[2]()
                else:
                    rest.append(f)
            fillers[:] = rest

        for i in range(NLQ):
            force_due(0, i)
            for kind, pe_us, act_us, fn in attn_units(0, i):
                fn()
                credit += act_us - pe_us
                while credit > 0.2 and fillers:
                    pop_filler()
        # pair1 attention; leftover proj fillers first, then output
        # projection l-blocks as their ot chunks (both pairs) complete.
        credit = 0.0
        for i in range(NLQ):
            force_due(1, i)
            for kind, pe_us, act_us, fn in attn_units(1, i):
                fn()
                if isinstance(kind, tuple) and kind[0] == "pv":
                    for hf in range(2):
                        fillers.append(
                            (0.43, None,
                             lambda ll=4 * i + kind[1], h=hf:
                                 outproj_half(ll, h)))
                    if i == NLQ - 1:
                        # keep the tail short: drain outproj during the
                        # last i-block regardless of ACT credit
                        for _ in range(5):
                            if len(fillers) > 2:
                                pop_filler()
                credit += act_us - pe_us
                # reserve a few outproj groups so the last i-block's exp
                # backlog (no new fillers arrive there) still has PE cover
                reserve = 18 if i < NLQ - 1 else 0
                while credit > 0.2 and len(fillers) > reserve:
                    pop_filler()
        for _pe, _need, f_fn in fillers:
            f_fn()

    nc.compile()
    return nc


_CACHE = {}


def _get_nc(mm_dt=None, iters=1):
    key = (str(mm_dt), iters)
    if key not in _CACHE:
        _CACHE[key] = build_module(mm_dt, iters)
    return _CACHE[key]


def _np_fp8():
    import ml_dtypes
    return ml_dtypes.float8_e4m3


def _np_bf16():
    import ml_dtypes
    return ml_dtypes.bfloat16


def _make_in_maps(x, causal_mask, wq, bq, wk, bk, wv, bv, wo):
    fp8 = _np_fp8()
    bf16 = _np_bf16()
    x = np.asarray(x, np.float32)
    cm = np.asarray(causal_mask)
    # 4 multiplicative mask tiles [128, 512]: tile m covers k-block j=4i+m
    # within Lq-block i -> tile[p, c] = 0 if mask(q=c, k=128m+p) else 1
    mt = np.empty((4, P, LQB), np.float32)
    for m in range(4):
        mt[m] = (~cm[0, 0, 0:LQB, m * P:(m + 1) * P]).T.astype(np.float32)
    mt = mt.astype(bf16)
    ident = np.eye(P, dtype=bf16)

    # fp8 e4m3 has its min normal at 2^-6; scale x by 8 and w by 32 so the
    # hi/lo residuals stay out of the subnormal floor (verified: unscaled
    # 3-term compensation is ~3e-2, scaled ~1.3e-3). The combined 256x is
    # folded into bq/bk (x256), the exp scale (/65536), and wo (/256).
    def split8(a, s):
        a = np.ascontiguousarray(a) * s
        hi = a.astype(fp8)
        lo = (a - hi.astype(np.float32)).astype(fp8)
        return hi, lo

    def warr(w):
        # [D, 2P] -> [P, KK, 2, 2P]
        return np.ascontiguousarray(
            w.reshape(KK, 2, P, 2 * P).transpose(2, 0, 1, 3))

    in_maps = []
    for c in range(N_CORES):
        b = c // 4
        g = c % 4
        cols = slice(2 * P * g, 2 * P * (g + 1))
        xt = np.ascontiguousarray(x[b].T)
        xt_hi, xt_lo = split8(xt, 8.0)
        m = {"xt_hi": xt_hi, "xt_lo": xt_lo,
             "bq": np.ascontiguousarray(np.asarray(bq, np.float32)[cols]) * 256.0,
             "bk": np.ascontiguousarray(np.asarray(bk, np.float32)[cols]) * 256.0,
             "wo": np.ascontiguousarray(
                 np.asarray(wo, np.float32)[cols, :] / 256.0).astype(bf16),
             "mask": mt, "ident": ident}
        for wn, w in (("wq", wq), ("wk", wk), ("wv", wv)):
            wc = np.asarray(w, np.float32)[:, cols]
            hi, lo = split8(wc, 32.0)
            m[f"{wn}_hi"] = warr(hi)
            m[f"{wn}_lo"] = warr(lo)
        in_maps.append(m)
    return in_maps


def run(inputs, trace=False, mm_dt=None, iters=1, **kw):
    nc = _get_nc(mm_dt, iters)
    in_maps = _make_in_maps(
        inputs["x"], inputs["causal_mask"], inputs["wq"], inputs["bq"],
        inputs["wk"], inputs["bk"], inputs["wv"], inputs["bv"], inputs["wo"])
    res = run_bass_kernel_spmd(nc, in_maps, list(range(N_CORES)),
                               trace=trace, **kw)
    bo = np.asarray(inputs["bo"], np.float32)
    bv = np.asarray(inputs["bv"], np.float32)
    wo_f = np.asarray(inputs["wo"], np.float32)
    out = np.zeros((B, L, D), np.float32)
    for c in range(N_CORES):
        out[c // 4] += res.results[c]["out"].astype(np.float32)
    out += (bo + bv @ wo_f)[None, None, :]
    return out, res


def kernel(**inputs):
    out, _ = run(inputs)
    return out
